# revision 45
# baseline (speedup 1.0000x reference)
"""Trainium2 Bass kernel for nn_MultiHeadAttention_37512244363503.

Sharding: 8 cores = 4 batches x 2 head-groups (8 heads each).
Per core (b, g): Wq/Wk/Wv column-sliced, Wo row-sliced; the host sums the
two partial outputs per batch (the row-parallel "all-reduce") and adds bo.

Per-core algorithm (all matmuls bf16 in / fp32 PSUM accumulate):
  QT[d,i]   = Wq_g.T @ x_q[b].T         (d=512 cols of this group, i=1024)
  KT[d,j]   = Wk_g.T @ x_kv[b].T
  V[j,c]    = x_kv[b] @ Wv_g            (per-head [V_h | ones*64] blocks)
  ST[j,i]   = K_h Q_h.T   per head      (scores transposed: j on partitions,
                                         2 heads packed via PE row groups)
  PT        = exp(ST/8 + mask_bias[j])  (pad mask rides the ACT bias input)
  po        = V_ext.T @ PT              (rows 0:64 = OT, 64:128 = denom)
  O_norm^T  = po[0:64] * 1/po[64:128]   (cross-base reciprocal + mul)
  partial   = O_norm @ Wo_g             (PSUM -> SBUF -> DRAM)
"""

import numpy as np
import ml_dtypes

import concourse.bass as bass
import concourse.mybir as mybir
from concourse import bacc
from concourse.tile import TileContext
from concourse.bass_utils import run_bass_kernel_spmd

BF16 = ml_dtypes.bfloat16

B, N, L, D, H = 4, 1024, 1024, 1024, 16
DH = D // H           # 64 channels per head
HG = 8                # heads per core
DG = HG * DH          # 512 channels per core
NCORES = 8
DP_SCALE = DH ** -0.5
MASK_NEG = -30000.0   # exp(x + MASK_NEG) underflows to exactly 0.0 in fp32

f32 = mybir.dt.float32
bf16 = mybir.dt.bfloat16

KT_TILES = D // 128   # 8 k-tiles in the contraction dim of projections
JT = L // 128         # 8 key tiles
IT = N // 128         # 8 query tiles
IC = N // 512         # 2 query chunks (PSUM free dim)
PAIRS = HG // 2       # 4 head pairs (2 heads packed per 128 partitions)
VW = 2 * DH           # 128 cols per (j, head) V_ext block: [V_h | ones]


def build_nc(debug=False, num_devices=NCORES, repeat=1):
    nc = bacc.Bacc("TRN2", target_bir_lowering=False, debug=False,
                   num_devices=num_devices)

    xqT = nc.dram_tensor("xqT", [D, N], bf16, kind="ExternalInput")
    xkvT = nc.dram_tensor("xkvT", [D, L], bf16, kind="ExternalInput")
    wq = nc.dram_tensor("wq", [D, DG], bf16, kind="ExternalInput")
    wk = nc.dram_tensor("wk", [D, DG], bf16, kind="ExternalInput")
    wv = nc.dram_tensor("wv", [D, DG], bf16, kind="ExternalInput")
    wo = nc.dram_tensor("wo", [DG, D], bf16, kind="ExternalInput")
    mb = nc.dram_tensor("mb", [128, JT], f32, kind="ExternalInput")
    out = nc.dram_tensor("out", [N, D], f32, kind="ExternalOutput")
    dbg = {}
    if debug:
        dbg["d_qT"] = nc.dram_tensor("d_qT", [128, PAIRS * N], bf16,
                                     kind="ExternalOutput")
        dbg["d_kT"] = nc.dram_tensor("d_kT", [128, PAIRS * L], bf16,
                                     kind="ExternalOutput")
        dbg["d_v"] = nc.dram_tensor("d_v", [128, JT * HG * VW], bf16,
                                    kind="ExternalOutput")
        dbg["d_ot"] = nc.dram_tensor("d_ot", [128, PAIRS * N], bf16,
                                     kind="ExternalOutput")

    with TileContext(nc) as tc:
        with (
            tc.tile_pool(name="persist", bufs=1) as persist,
            tc.tile_pool(name="pt", bufs=2) as pt_pool,
            tc.tile_pool(name="recp", bufs=4) as recp,
            tc.tile_pool(name="scp", bufs=4) as scp,
            tc.tile_pool(name="stps", bufs=2, space="PSUM") as stps,
            tc.tile_pool(name="smps", bufs=4, space="PSUM") as smps,
        ):
            # ---- persistent SBUF tensors ----
            xq_sb = persist.tile([128, KT_TILES * N], bf16)       # (k, i)
            xkv_sb = persist.tile([128, KT_TILES * L], bf16)      # (k, j)
            wq_sb = persist.tile([128, KT_TILES * DG], bf16)      # (k, d)
            wk_sb = persist.tile([128, KT_TILES * DG], bf16)
            wv_sb = persist.tile([128, KT_TILES * DG], bf16)
            wo_sb = persist.tile([128, PAIRS * D], bf16)          # (ctile, d)
            qT_sb = persist.tile([128, PAIRS * N], bf16)          # (pair, i)
            kT_sb = persist.tile([128, PAIRS * L], bf16)          # (pair, j)
            v_sb = persist.tile([128, JT * HG * VW], bf16)        # (j, h, 2c)
            ot_sb = persist.tile([128, PAIRS * N], bf16)          # (pair, i)
            mb_sb = persist.tile([128, JT], f32)

            # ones half of each V_ext block (cols DH..2*DH-1 of each block)
            v_view = v_sb[:].rearrange("p (j h c) -> p j h c", j=JT, h=HG)
            nc.vector.memset(v_view[:, :, :, DH:VW], 1.0)

            for _rep in range(repeat):
                _emit_body(nc, tc, locals())

    nc.compile()
    return nc


def _emit_body(nc, tc, env):
    xq_sb, xkv_sb = env["xq_sb"], env["xkv_sb"]
    wq_sb, wk_sb, wv_sb, wo_sb = (env[k] for k in
                                  ["wq_sb", "wk_sb", "wv_sb", "wo_sb"])
    qT_sb, kT_sb, v_sb, ot_sb, mb_sb = (env[k] for k in
                                        ["qT_sb", "kT_sb", "v_sb", "ot_sb",
                                         "mb_sb"])
    v_view = env["v_view"]
    pt_pool, recp, scp = env["pt_pool"], env["recp"], env["scp"]
    stps, smps = env["stps"], env["smps"]
    out, debug, dbg = env["out"], env["debug"], env["dbg"]
    xqT, xkvT, wq, wk, wv, wo, mb = (env[k] for k in
                                     ["xqT", "xkvT", "wq", "wk", "wv", "wo",
                                      "mb"])

    if True:
        if True:
            # ---- input loads ----
            # Two HWDGE queues (sync + scalar), few big DMAs, proj-0 critical
            # path first. DRAM views are rearranged so one DMA fills the
            # k-tile-major SBUF layout.
            def load(eng, dst_sb, src, blk, part, nk=4):
                rows = slice(part * nk * 128, (part * nk + nk) * 128)
                cols = slice(part * nk * blk, (part * nk + nk) * blk)
                eng.dma_start(
                    out=dst_sb[:, cols].rearrange("p (k c) -> p k c", k=nk),
                    in_=src[rows, :].rearrange("(k p) c -> p k c", p=128))

            nc.gpsimd.dma_start(out=mb_sb[:], in_=mb[:, :])
            # arrival-ordered across the 3 DMA queues: proj-0's QT needs
            # xq+wq first, KT needs xkv+wk next, V's wv after that.
            for q in range(4):
                load(nc.sync, xq_sb, xqT, N, q, nk=2)
                load(nc.scalar, wq_sb, wq, DG, q, nk=2)
            load(nc.gpsimd, xkv_sb, xkvT, L, 0)
            load(nc.sync, xkv_sb, xkvT, L, 1)
            for q in range(4):
                load(nc.scalar, wk_sb, wk, DG, q, nk=2)
            load(nc.gpsimd, wv_sb, wv, DG, 0)
            load(nc.scalar, wv_sb, wv, DG, 1)
            nc.sync.dma_start(
                out=wo_sb[:].rearrange("p (ct d) -> p ct d", ct=PAIRS),
                in_=wo[:, :].rearrange("(ct p) d -> p ct d", p=128))

            def v_proj(j):
                """V[j, c] = x_kv @ Wv_g for one j tile."""
                ps = smps.tile([128, 512], f32, tag="ps")
                for k in range(KT_TILES):
                    nc.tensor.matmul(
                        ps[:],
                        lhsT=xkv_sb[:, k * L + j * 128: k * L + (j + 1) * 128],
                        rhs=wv_sb[:, k * DG:(k + 1) * DG],
                        start=(k == 0), stop=(k == KT_TILES - 1))
                # scatter heads into the VW-col-stride layout
                nc.vector.tensor_copy(
                    out=v_view[:, j, :, 0:DH],
                    in_=ps[:].rearrange("p (h c) -> p h c", h=HG))

            def project_half(dst_sb, w_sb, x_sb, p, ic):
                """one i-chunk of a projection for pair p"""
                ps = smps.tile([128, 512], f32, tag="ps")
                for k in range(KT_TILES):
                    nc.tensor.matmul(
                        ps[:],
                        lhsT=w_sb[:, k * DG + p * 128: k * DG + (p + 1) * 128],
                        rhs=x_sb[:, k * N + ic * 512: k * N + ic * 512 + 512],
                        start=(k == 0), stop=(k == KT_TILES - 1))
                nc.vector.tensor_copy(
                    out=dst_sb[:, p * N + ic * 512: p * N + ic * 512 + 512],
                    in_=ps[:])

            def project(dst_sb, w_sb, x_sb, p):
                for ic in range(IC):
                    project_half(dst_sb, w_sb, x_sb, p, ic)

            def st_pair_flash(p, pa, pb, filler):
                """ST + exp for pair p with filler(j) PE work per j slot."""
                for j in range(JT):
                    psa = stps.tile([128, 1024], f32, tag="st")
                    psb = stps.tile([128, 1024], f32, tag="st")
                    for ic in range(IC):
                        cols = slice(ic * 512, ic * 512 + 512)
                        nc.tensor.matmul(
                            psa[:, cols],
                            lhsT=kT_sb[0:64, p * L + j * 128: p * L + (j + 1) * 128],
                            rhs=qT_sb[0:64, p * N + ic * 512: p * N + ic * 512 + 512],
                            start=True, stop=True)
                        nc.tensor.matmul(
                            psb[:, cols],
                            lhsT=kT_sb[64:128, p * L + j * 128: p * L + (j + 1) * 128],
                            rhs=qT_sb[64:128, p * N + ic * 512: p * N + ic * 512 + 512],
                            start=True, stop=True)
                    nc.scalar.activation(
                        out=pa[:, j * N:(j + 1) * N], in_=psa[:],
                        func=mybir.ActivationFunctionType.Exp,
                        bias=mb_sb[:, j:j + 1], scale=DP_SCALE)
                    nc.scalar.activation(
                        out=pb[:, j * N:(j + 1) * N], in_=psb[:],
                        func=mybir.ActivationFunctionType.Exp,
                        bias=mb_sb[:, j:j + 1], scale=DP_SCALE)
                    filler(j)

            def norm(po, p, hh, ic):
                rec_t = recp.tile([64, 512], f32)
                nc.vector.reciprocal(out=rec_t[:], in_=po[64:128, :])
                sc_t = scp.tile([64, 512], bf16, tag="sc")
                nc.vector.tensor_mul(out=sc_t[:], in0=po[0:64, :], in1=rec_t[:])
                dst = p * N + ic * 512
                nc.sync.dma_start(
                    out=ot_sb[hh * 64:(hh + 1) * 64, dst:dst + 512],
                    in_=sc_t[:])

            def av_burst_half(p, hh, pt, ic):
                """AV for head 2p+hh, one i chunk, from materialized pt."""
                h = 2 * p + hh
                po = smps.tile([128, 512], f32, tag="ps")
                for j in range(JT):
                    base = (j * HG + h) * VW
                    nc.tensor.matmul(
                        po[:],
                        lhsT=v_sb[:, base: base + VW],
                        rhs=pt[:, j * N + ic * 512: j * N + ic * 512 + 512],
                        start=(j == 0), stop=(j == JT - 1))
                norm(po, p, hh, ic)

            # ---- pipelined pairs ----
            # pair 0 interleaves the V projection into its j-loop; pairs
            # p>0 interleave the (p+1) projections. Head B's AV runs as a
            # burst at each pair boundary, overlapping the next pair's loop.
            project(qT_sb, wq_sb, xq_sb, 0)
            project(kT_sb, wk_sb, xkv_sb, 0)
            prev = None
            for p in range(PAIRS):
                pa = pt_pool.tile([128, JT * N], bf16, tag="pa")
                pb = pt_pool.tile([128, JT * N], bf16, tag="pb")

                # filler units for this pair's j slots: the next pair's
                # projections interleaved with the previous pair's AV bursts
                units = []
                if p + 1 < PAIRS:
                    units += [lambda pp=p: project_half(qT_sb, wq_sb, xq_sb, pp + 1, 0),
                              lambda pp=p: project_half(qT_sb, wq_sb, xq_sb, pp + 1, 1),
                              lambda pp=p: project_half(kT_sb, wk_sb, xkv_sb, pp + 1, 0),
                              lambda pp=p: project_half(kT_sb, wk_sb, xkv_sb, pp + 1, 1)]
                if prev is not None:
                    pp, ppa, ppb = prev
                    units += [lambda: av_burst_half(pp, 0, ppa, 0),
                              lambda: av_burst_half(pp, 0, ppa, 1),
                              lambda: av_burst_half(pp, 1, ppb, 0),
                              lambda: av_burst_half(pp, 1, ppb, 1)]
                # spread the units across the 8 j slots, alternating kinds
                order = [0, 4, 1, 5, 2, 6, 3, 7] if len(units) == 8 else list(range(len(units)))
                slot_units = {}
                for s, u in enumerate(order):
                    slot_units[s + (JT - len(order))] = units[u]

                if p == 0:
                    def filler(j, slot_units=slot_units):
                        v_proj(j)
                        if j in slot_units:
                            slot_units[j]()
                else:
                    def filler(j, slot_units=slot_units):
                        if j in slot_units:
                            slot_units[j]()
                st_pair_flash(p, pa, pb, filler)
                prev = (p, pa, pb)
            # last pair's AV bursts + remaining norm work
            pp, ppa, ppb = prev
            for hh, pt in ((0, ppa), (1, ppb)):
                for ic in range(IC):
                    av_burst_half(pp, hh, pt, ic)


            if debug:
                nc.sync.dma_start(out=dbg["d_qT"][:, :], in_=qT_sb[:])
                nc.sync.dma_start(out=dbg["d_kT"][:, :], in_=kT_sb[:])
                nc.sync.dma_start(out=dbg["d_v"][:, :], in_=v_sb[:])
                nc.sync.dma_start(out=dbg["d_ot"][:, :], in_=ot_sb[:])

            # ---- output projection: partial[i, d] ----
            for it in range(IT):
                for dc in range(IC):
                    ps = smps.tile([128, 512], f32, tag="ps")
                    for ct in range(PAIRS):
                        nc.tensor.matmul(
                            ps[:],
                            lhsT=ot_sb[:, ct * N + it * 128: ct * N + (it + 1) * 128],
                            rhs=wo_sb[:, ct * D + dc * 512: ct * D + dc * 512 + 512],
                            start=(ct == 0), stop=(ct == PAIRS - 1))
                    out_t = scp.tile([128, 512], f32, tag="outt")
                    nc.vector.tensor_copy(out=out_t[:], in_=ps[:])
                    nc.sync.dma_start(
                        out=out[it * 128:(it + 1) * 128, dc * 512: dc * 512 + 512],
                        in_=out_t[:])


_NC = None


def _get_nc():
    global _NC
    if _NC is None:
        _NC = build_nc()
    return _NC


def _make_in_maps(x_q, x_kv, pad_mask, Wq, Wk, Wv, Wo):
    in_maps = []
    for c in range(NCORES):
        b, g = c // 2, c % 2
        cols = slice(g * DG, (g + 1) * DG)
        mbias = np.where(pad_mask[b], np.float32(MASK_NEG), np.float32(0.0))
        in_maps.append({
            "xqT": np.ascontiguousarray(x_q[b].T).astype(BF16),
            "xkvT": np.ascontiguousarray(x_kv[b].T).astype(BF16),
            "wq": np.ascontiguousarray(Wq[:, cols]).astype(BF16),
            "wk": np.ascontiguousarray(Wk[:, cols]).astype(BF16),
            "wv": np.ascontiguousarray(Wv[:, cols]).astype(BF16),
            "wo": np.ascontiguousarray(Wo[g * DG:(g + 1) * DG, :]).astype(BF16),
            "mb": np.ascontiguousarray(
                mbias.astype(np.float32).reshape(JT, 128).T),
        })
    return in_maps


def kernel(x_q, x_kv, pad_mask, Wq, Wk, Wv, Wo, bo):
    nc = _get_nc()
    in_maps = _make_in_maps(x_q, x_kv, pad_mask, Wq, Wk, Wv, Wo)
    res = run_bass_kernel_spmd(nc, in_maps, core_ids=list(range(NCORES)))
    full = np.empty((B, N, D), dtype=np.float32)
    for b in range(B):
        full[b] = res.results[2 * b]["out"] + res.results[2 * b + 1]["out"]
        full[b] += bo.astype(np.float32)
    return full
